# revision 61
# baseline (speedup 1.0000x reference)
"""NonLocalBlock (GroupNorm + single-head 4096x4096 attention + residual)
Trainium2 Bass kernel, data-parallel over batch: 1 image per NeuronCore x8.

Per image (x: [512, 4096] channels-major):
  pass0: GroupNorm stats (bn_stats per channel, group-combine via tiny matmuls)
  passA: per hw-chunk of 512: normalize -> h (fp32r), h16 = fp16(h) resident,
         kk = (Wq^T Wk) @ h via fp32r 1-pass (M precomputed on host in fp64),
         kk16 resident, vT (fp32r 1-pass) -> fp16 resident.
  attention: logits = h16^T @ kk16 (1-pass fp16 matmuls straight into PSUM,
         all 8 chunks of a q-tile live in PSUM banks), row max on DVE from
         PSUM, ACT exp reads PSUM -> fp16 probs (+exact fp32 row sums),
         fp16 PE-transpose probs, fp16 attn@v, fp16 transpose attn_h (+bv),
         fp16 output projection, +bo +residual fused on DVE, store.
  q is never materialized: softmax(q^T k) == softmax(h^T (Wq^T Wk) h); the
  q-row bias term q^T bk is softmax-invariant; bq column term handled in the
  with_qk_bias build variant (bq/bk are zero in practice).
"""
import sys

sys.path.insert(0, '/opt/trn_rl_repo')
import numpy as np
import concourse.bass as bass
import concourse.bacc as bacc
import concourse.mybir as mybir
import concourse.tile as tile
from concourse.bass_utils import run_bass_kernel_spmd

F32 = mybir.dt.float32
F32R = mybir.dt.float32r
F16 = mybir.dt.float16
AF = mybir.ActivationFunctionType
AX = mybir.AxisListType
OP = mybir.AluOpType

C = 512
HW = 4096
NT = 4            # channel tiles of 128
NCH = 8           # hw chunks of 512
NQT = 32          # q tiles of 128
GSIZE = 16        # channels per group
EPS = 1e-5
SCALE = float(np.float32(512.0) ** 0.5)


def build(with_qk_bias=False):
    nc = bacc.Bacc('TRN2', target_bir_lowering=False, debug=False)

    x_in = nc.declare_dram_parameter("x", [C, HW], F32, isOutput=False)
    mT_in = nc.declare_dram_parameter("mT", [C, C], F32, isOutput=False)
    wvT_in = nc.declare_dram_parameter("wvT", [C, C], F32, isOutput=False)
    wo16_in = nc.declare_dram_parameter("woT16", [C, C], F16, isOutput=False)
    bias_in = nc.declare_dram_parameter("biases", [128, 16], F32,
                                        isOutput=False)  # bq|bk|bv|bo as [128,4]
    gb_in = nc.declare_dram_parameter("gammabeta", [128, 8], F32,
                                      isOutput=False)  # gamma|beta as [128,4]
    if with_qk_bias:
        u_in = nc.declare_dram_parameter("uT", [C, 1], F32, isOutput=False)
    out_dram = nc.declare_dram_parameter("out", [C, HW], F32, isOutput=True)

    a16 = np.zeros((128, 8), np.float32)
    for p in range(128):
        a16[p, p // GSIZE] = 1.0 / GSIZE
    b8 = np.zeros((8, 128), np.float32)
    for p in range(128):
        b8[p // GSIZE, p] = 1.0
    a16_d = nc.inline_tensor(a16, "a16")
    b8_d = nc.inline_tensor(b8, "b8")
    id16_d = nc.inline_tensor(np.eye(128, dtype=np.float16), "ident16")
    ones_d = nc.inline_tensor(np.ones((1, 128), np.float32), "ones128")

    with tile.TileContext(nc) as tc:
        with (
            tc.tile_pool(name="res", bufs=1) as res,
            tc.tile_pool(name="pp_log", bufs=3, space="PSUM") as pp_log,
            tc.tile_pool(name="pp_t", bufs=2, space="PSUM") as pp_t,
            tc.tile_pool(name="pp_at", bufs=1, space="PSUM") as pp_at,
            tc.tile_pool(name="pp_o", bufs=2, space="PSUM") as pp_o,
        ):
            # ---------- residents ----------
            h16_res = [res.tile([128, HW], F16, tag=f"h16_{t}", name=f"h16_{t}")
                       for t in range(NT)]
            kk16_res = [res.tile([128, HW], F16, tag=f"kk{t}", name=f"kk{t}")
                        for t in range(NT)]
            vT_res = [res.tile([128, C], F16, tag=f"vT{m}", name=f"vT{m}")
                      for m in range(NQT)]
            mT_sb = [res.tile([128, C], F32R, tag=f"mT{t}", name=f"mT{t}")
                     for t in range(NT)]
            wv_sb = [res.tile([128, C], F32R, tag=f"wv{t}", name=f"wv{t}")
                     for t in range(NT)]
            wo_sb = [res.tile([128, C], F16, tag=f"wo{t}", name=f"wo{t}")
                     for t in range(NT)]
            id16_sb = res.tile([128, 128], F16, tag="ident16")
            nc.sync.dma_start(out=id16_sb, in_=id16_d[:])
            # warm-up transposes don't need real identities -- a memset tile
            # is ready ~1us in, long before any DMA lands
            wdata = res.tile([128, 128], F32, tag="wdata")
            nc.vector.memset(wdata, 1.0)
            # mT/wv need the gpsimd casting DMA (fp32 -> fp32r rounding);
            # pass0's x loads go on sync/scalar so these don't delay them
            for t in range(NT):
                sl = slice(128 * t, 128 * (t + 1))
                nc.gpsimd.dma_start(out=mT_sb[t], in_=mT_in[sl, :])
                nc.gpsimd.dma_start(out=wv_sb[t], in_=wvT_in[sl, :])
            biases = res.tile([128, 16], F32, tag="biases")
            nc.sync.dma_start(out=biases, in_=bias_in[:])
            bv = biases[:, 8:12]
            bo = biases[:, 12:16]
            gmbt = res.tile([128, 8], F32, tag="gmbt")
            nc.sync.dma_start(out=gmbt, in_=gb_in[:])
            gam = gmbt[:, 0:4]
            bet = gmbt[:, 4:8]
            a16_sb = res.tile([128, 8], F32, tag="a16")
            nc.sync.dma_start(out=a16_sb, in_=a16_d[:])
            b8_sb = res.tile([8, 128], F32, tag="b8")
            nc.sync.dma_start(out=b8_sb, in_=b8_d[:])
            if with_qk_bias:
                u_sb = res.tile([128, NT], F32R, tag="u_sb")
                nc.gpsimd.dma_start(
                    out=u_sb, in_=u_in[:].rearrange("(t p) o -> p (t o)", p=128))
                ones_col = res.tile([1, 128], F32R, tag="ones_col")
                nc.gpsimd.dma_start(out=ones_col, in_=ones_d[:])
            eps8 = res.tile([8, 1], F32, tag="eps8")
            nc.vector.memset(eps8, EPS)
            scale_sb = res.tile([128, NT], F32, tag="scale")
            shift_sb = res.tile([128, NT], F32, tag="shift")

            wps = pp_log.tile([128, 512], F32, tag="ps_l", name="wps")
            # ---------- pass 0: GroupNorm statistics ----------
            # x chunks 0-3 (hw 0-2047) stay resident after pass0: passA and
            # the group 0-3 residual adds reuse them instead of re-reading HBM
            n_persist = 0 if with_qk_bias else 2
            xp = [[res.tile([128, 1024], F32, tag=f"xp{t}_{n}",
                            name=f"xp{t}_{n}") for n in range(n_persist)]
                  for t in range(NT)]
            with tc.tile_pool(name="p0", bufs=4) as p0, \
                 tc.tile_pool(name="p0s", bufs=1) as p0s:
                st6 = p0s.tile([128, NT, NCH, 6], F32, tag="st6")
                # PE warmup: dummy transposes through pass0 keep HAM
                # unthrottled (1.2->2.4GHz) until passA matmuls start.
                # 512-wide ones bridge the ~10us before the first x lands.
                for _ in range(110):
                    nc.tensor.transpose(wps[:, 0:128], wdata, wdata)
                for n in range(4):
                    for t in range(NT):
                        if n < n_persist:
                            xc = xp[t][n]
                        else:
                            xc = p0.tile([128, 1024], F32, tag="x0")
                        eng = nc.sync if (t % 2 == 0) else nc.scalar
                        eng.dma_start(
                            out=xc,
                            in_=x_in[128 * t:128 * (t + 1),
                                     1024 * n:1024 * (n + 1)])
                        nc.vector.bn_stats(out=st6[:, t, 2 * n, :],
                                           in_=xc[:, 0:512])
                        nc.vector.bn_stats(out=st6[:, t, 2 * n + 1, :],
                                           in_=xc[:, 512:1024])
                        # keep-warm: depends on xc's DMA, so it lands mid-pass0
                        for _ in range(6):
                            nc.tensor.transpose(wps[:, 0:128], xc[:, 0:128], wdata)
                # wo16 isn't needed until the first outproj (~200us in)
                for t in range(NT):
                    nc.sync.dma_start(out=wo_sb[t],
                                      in_=wo16_in[128 * t:128 * (t + 1), :])
                mv = p0s.tile([128, NT, 2], F32, tag="mv")
                for t in range(NT):
                    nc.vector.bn_aggr(out=mv[:, t, :], in_=st6[:, t, :, :])
                # stats_in: cols 0-3 mean_t, cols 4-7 E[x^2]_t
                stats_in = p0s.tile([128, 8], F32, tag="stats_in")
                for t in range(NT):
                    nc.vector.tensor_copy(stats_in[:, t:t + 1], mv[:, t, 0:1])
                    nc.vector.tensor_mul(stats_in[:, 4 + t:5 + t],
                                         mv[:, t, 0:1], mv[:, t, 0:1])
                    nc.vector.tensor_add(stats_in[:, 4 + t:5 + t],
                                         stats_in[:, 4 + t:5 + t], mv[:, t, 1:2])
                for _ in range(4):
                    nc.tensor.transpose(wps[0:8, 0:128], stats_in, wdata)
                ps_g = pp_o.tile([8, 8], F32, tag="ps_o")
                nc.tensor.matmul(ps_g, a16_sb, stats_in, start=True, stop=True)
                g_sb = p0s.tile([8, 8], F32, tag="g_sb")
                nc.vector.tensor_copy(g_sb, ps_g)
                # group var = E[x^2]_g - mean_g^2 ; rstd = exp(-0.5*ln(var+eps))
                var_g = p0s.tile([8, 4], F32, tag="var_g")
                nc.vector.tensor_mul(var_g, g_sb[:, 0:4], g_sb[:, 0:4])
                nc.vector.tensor_tensor(out=var_g, in0=g_sb[:, 4:8], in1=var_g,
                                        op=OP.subtract)
                bc_in = p0s.tile([8, 8], F32, tag="bc_in")
                nc.vector.tensor_copy(bc_in[:, 0:4], g_sb[:, 0:4])
                nc.scalar.activation(out=bc_in[:, 4:8], in_=var_g, func=AF.Ln,
                                     bias=eps8, scale=1.0)
                nc.scalar.activation(out=bc_in[:, 4:8], in_=bc_in[:, 4:8],
                                     func=AF.Exp, bias=0.0, scale=-0.5)
                ps_bc = pp_o.tile([128, 8], F32, tag="ps_o")
                nc.tensor.matmul(ps_bc, b8_sb, bc_in, start=True, stop=True)
                chan = p0s.tile([128, 8], F32, tag="chan")
                nc.vector.tensor_copy(chan, ps_bc)
                for _ in range(4):
                    nc.tensor.transpose(wps[0:8, 0:128], chan, wdata)
                # scale = gamma * rstd ; shift = beta - mean*scale
                nc.vector.tensor_mul(scale_sb, gam, chan[:, 4:8])
                tmp = p0s.tile([128, NT], F32, tag="tmp")
                nc.vector.tensor_mul(tmp, chan[:, 0:4], scale_sb)
                nc.vector.tensor_tensor(out=shift_sb, in0=bet, in1=tmp,
                                        op=OP.subtract)

            # ---------- pass A: hidden -> h16, kk16, vT16 (+u row) ----------
            with tc.tile_pool(name="pa_x", bufs=8) as pa_x, \
                 tc.tile_pool(name="pa_hr", bufs=8) as pa_hr:
                if with_qk_bias:
                    r_row = res.tile([1, HW], F32R, tag="r_row")
                for n in range(NCH):
                    cols = slice(512 * n, 512 * (n + 1))
                    hid_r = []
                    for t in range(NT):
                        if n < 2 * n_persist:
                            xc = xp[t][n // 2][:, 512 * (n % 2):
                                               512 * (n % 2) + 512]
                        else:
                            xc = pa_x.tile([128, 512], F32, tag="xA")
                            eng = nc.sync if (t % 2 == 0) else nc.scalar
                            eng.dma_start(
                                out=xc, in_=x_in[128 * t:128 * (t + 1), cols])
                        hr = pa_hr.tile([128, 512], F32R, tag="hid_r", bufs=8)
                        nc.vector.tensor_scalar(
                            out=hr, in0=xc,
                            scalar1=scale_sb[:, t:t + 1],
                            scalar2=shift_sb[:, t:t + 1],
                            op0=OP.mult, op1=OP.add)
                        hid_r.append(hr)
                        nc.scalar.copy(out=h16_res[t][:, cols], in_=hr)
                    # kk = M @ h (fp32r 1-pass), round to fp16
                    for t in range(NT):
                        ps = pp_o.tile([128, 512], F32, tag="ps_o")
                        for kc in range(NT):
                            nc.tensor.matmul(
                                ps, mT_sb[kc][:, 128 * t:128 * (t + 1)],
                                hid_r[kc], start=(kc == 0), stop=(kc == 3))
                        nc.scalar.copy(out=kk16_res[t][:, cols], in_=ps)
                    # vT (fp32r 1-pass): out[hw_t 128, c 512], round to fp16
                    # (bv folded into attn_h later: softmax weights sum to 1)
                    for t in range(NT):
                        ps = pp_o.tile([128, 512], F32, tag="ps_o")
                        for kc in range(NT):
                            nc.tensor.matmul(
                                ps, hid_r[kc][:, 128 * t:128 * (t + 1)],
                                wv_sb[kc], start=(kc == 0), stop=(kc == 3))
                        if t % 2 == 0:
                            nc.vector.tensor_copy(vT_res[4 * n + t], ps)
                        else:
                            nc.scalar.copy(out=vT_res[4 * n + t], in_=ps)
                    if with_qk_bias:
                        # r = u^T h  [1, 512] chunk (bq column term)
                        ps_r = pp_log.tile([1, 512], F32, tag="ps_l")
                        for kc in range(NT):
                            nc.tensor.matmul(
                                ps_r, u_sb[:, kc:kc + 1], hid_r[kc],
                                start=(kc == 0), stop=(kc == 3))
                        nc.vector.tensor_copy(r_row[:, cols], ps_r)

            # ---------- attention (software-pipelined over q-tiles) ----------
            # stage qt:   logits matmuls -> PSUM, chunk maxes, stage to SBUF
            # stage qt-1: softmax tail: exp -> fp16 probs -> transpose -> attn@v
            # stage qt-2: attn_h transpose + bv add into the group buffer
            # Issuing the tails AFTER the next q-tile's logits keeps the
            # in-order Tensor queue from stalling on the ACT exp latency.
            with tc.tile_pool(name="at_l", bufs=2) as at_l, \
                 tc.tile_pool(name="at_p", bufs=2) as at_p, \
                 tc.tile_pool(name="at_pt", bufs=2) as at_pt, \
                 tc.tile_pool(name="at_s", bufs=2) as at_s, \
                 tc.tile_pool(name="at_h4", bufs=2) as at_h4, \
                 tc.tile_pool(name="at_o", bufs=2) as at_o:
                lgs, maxss, attns, h4s, xress = {}, {}, {}, {}, {}

                def logits_stage(qt):
                    qcols = slice(128 * qt, 128 * (qt + 1))
                    lg = at_l.tile([128, HW], F32, tag="lg")
                    maxs = at_s.tile([128, NCH], F32, tag="maxs")
                    for n in range(NCH):
                        ncols = slice(512 * n, 512 * (n + 1))
                        ps_l = pp_log.tile([128, 512], F32, tag="ps_l")
                        for kc in range(NT):
                            nc.tensor.matmul(
                                ps_l, h16_res[kc][:, qcols],
                                kk16_res[kc][:, ncols],
                                start=(kc == 0), stop=(kc == 3 and
                                                       not with_qk_bias))
                        if with_qk_bias:
                            nc.tensor.matmul(
                                ps_l, ones_col, r_row[:, ncols],
                                start=False, stop=True)
                        nc.vector.reduce_max(out=maxs[:, n:n + 1], in_=ps_l,
                                             axis=AX.X)
                        if n % 2 == 0:
                            nc.scalar.copy(out=lg[:, ncols], in_=ps_l)
                        else:
                            nc.vector.tensor_copy(out=lg[:, ncols], in_=ps_l)
                    lgs[qt], maxss[qt] = lg, maxs

                def softmax_av_stage(qt):
                    lg, maxs = lgs.pop(qt), maxss.pop(qt)
                    negmax = at_s.tile([128, 1], F32, tag="negmax")
                    nc.vector.reduce_max(out=negmax, in_=maxs, axis=AX.X,
                                         negate=True)
                    negmax_s = at_s.tile([128, 1], F32, tag="negmax_s")
                    nc.vector.tensor_scalar_mul(out=negmax_s, in0=negmax,
                                                scalar1=SCALE)
                    sums = at_s.tile([128, NCH], F32, tag="sums")
                    ps_at = pp_at.tile([128, C], F32, tag="ps_at")
                    for n in range(NCH):
                        probs = at_p.tile([128, 512], F16, tag="probs")
                        nc.scalar.activation(
                            out=probs, in_=lg[:, 512 * n:512 * (n + 1)],
                            func=AF.Exp, bias=negmax_s, scale=SCALE,
                            accum_out=sums[:, n:n + 1])
                        ps_t = pp_t.tile([128, 512], F16, tag="ps_t")
                        for j in range(4):
                            nc.tensor.transpose(
                                ps_t[:, 128 * j:128 * (j + 1)],
                                probs[:, 128 * j:128 * (j + 1)], id16_sb)
                        pT = at_pt.tile([128, 512], F16, tag="pT")
                        nc.vector.tensor_copy(pT, ps_t)
                        for j in range(4):
                            nc.tensor.matmul(
                                ps_at, pT[:, 128 * j:128 * (j + 1)],
                                vT_res[4 * n + j],
                                start=(n == 0 and j == 0),
                                stop=(n == 7 and j == 3))
                    rowsum = at_s.tile([128, 1], F32, tag="rowsum")
                    nc.vector.reduce_sum(out=rowsum, in_=sums, axis=AX.X)
                    rinv = at_s.tile([128, 1], F32, tag="rinv")
                    nc.vector.reciprocal(out=rinv, in_=rowsum)
                    attn = at_s.tile([128, C], F16, tag="attn")
                    nc.vector.tensor_scalar_mul(out=attn, in0=ps_at,
                                                scalar1=rinv)
                    attns[qt] = attn

                def attnh_stage(qt):
                    attn = attns.pop(qt)
                    attnh4 = h4s[qt // 4]
                    qq = qt % 4
                    ps_t2 = pp_t.tile([128, 512], F16, tag="ps_t")
                    for i in range(NT):
                        nc.tensor.transpose(
                            ps_t2[:, 128 * i:128 * (i + 1)],
                            attn[:, 128 * i:128 * (i + 1)], id16_sb)
                    for i in range(NT):
                        nc.vector.tensor_scalar_add(
                            out=attnh4[:, i, 128 * qq:128 * (qq + 1)],
                            in0=ps_t2[:, 128 * i:128 * (i + 1)],
                            scalar1=bv[:, i:i + 1])

                def outproj_stage(g):
                    attnh4 = h4s.pop(g)
                    gcols = slice(512 * g, 512 * (g + 1))
                    for m in range(NT):
                        ps_o = pp_o.tile([128, 512], F32, tag="ps_o")
                        for kc in range(NT):
                            nc.tensor.matmul(
                                ps_o, wo_sb[kc][:, 128 * m:128 * (m + 1)],
                                attnh4[:, kc, :], start=(kc == 0), stop=(kc == 3))
                        o_sb = at_o.tile([128, 512], F32, tag="o_sb", bufs=3)
                        nc.vector.scalar_tensor_tensor(
                            out=o_sb, in0=ps_o, scalar=bo[:, m:m + 1],
                            in1=xress[g][m], op0=OP.add, op1=OP.add)
                        eng = nc.sync if (m % 2 == 0) else nc.scalar
                        eng.dma_start(
                            out=out_dram[128 * m:128 * (m + 1), gcols], in_=o_sb)
                    del xress[g]

                for qt in range(NQT + 3):
                    if qt < NQT:
                        if qt % 4 == 0:
                            g = qt // 4
                            h4s[g] = at_h4.tile([128, NT, 512], F16,
                                                tag="attnh4", name=f"ah4_{g}")
                            if g < 2 * n_persist:
                                xress[g] = [
                                    xp[m][g // 2][:, 512 * (g % 2):
                                                  512 * (g % 2) + 512]
                                    for m in range(NT)]
                            else:
                                xres = [at_o.tile([128, 512], F32, tag="xres",
                                                  bufs=2, name=f"xres{g}_{m}")
                                        for m in range(NT)]
                                for m in range(NT):
                                    nc.sync.dma_start(
                                        out=xres[m],
                                        in_=x_in[128 * m:128 * (m + 1),
                                                 512 * g:512 * (g + 1)])
                                xress[g] = xres
                        logits_stage(qt)
                    if 1 <= qt <= NQT:
                        softmax_av_stage(qt - 1)
                    if qt >= 2 and qt - 2 < NQT:
                        attnh_stage(qt - 2)
                    if qt >= 3 and (qt - 3) % 4 == 3:
                        outproj_stage((qt - 3) // 4)

    nc.compile()
    return nc


_NC_CACHE = None
_NC_BIAS_CACHE = None


def _prep_inputs(inputs):
    x = np.asarray(inputs["x"], np.float32)

    def tile4(v):
        return np.asarray(v, np.float32).reshape(4, 128).T

    biases = np.concatenate(
        [tile4(inputs[k]) for k in ("bq", "bk", "bv", "bo")], axis=1)
    gb = np.concatenate(
        [tile4(inputs["gn_gamma"]), tile4(inputs["gn_beta"])], axis=1)
    wq = np.asarray(inputs["wq"], np.float64)
    wk = np.asarray(inputs["wk"], np.float64)
    mT = np.ascontiguousarray((wk.T @ wq).astype(np.float32))
    shared = {
        "mT": mT,
        "wvT": np.ascontiguousarray(np.asarray(inputs["wv"], np.float32).T),
        "woT16": np.ascontiguousarray(
            np.asarray(inputs["wo"], np.float32).T.astype(np.float16)),
        "biases": np.ascontiguousarray(biases),
        "gammabeta": np.ascontiguousarray(gb),
    }
    if np.any(np.asarray(inputs["bq"], np.float32)) or \
       np.any(np.asarray(inputs["bk"], np.float32)):
        u = wk.T @ np.asarray(inputs["bq"], np.float64)
        shared["uT"] = np.ascontiguousarray(
            u.astype(np.float32).reshape(C, 1))
    return [dict(shared, x=np.ascontiguousarray(x[i].reshape(C, HW)))
            for i in range(x.shape[0])]


def kernel(**inputs):
    global _NC_CACHE, _NC_BIAS_CACHE
    x = np.asarray(inputs["x"], np.float32)
    b, c, h, w = x.shape
    in_maps = _prep_inputs(inputs)
    if "uT" in in_maps[0]:
        if _NC_BIAS_CACHE is None:
            _NC_BIAS_CACHE = build(with_qk_bias=True)
        nc = _NC_BIAS_CACHE
    else:
        if _NC_CACHE is None:
            _NC_CACHE = build(with_qk_bias=False)
        nc = _NC_CACHE
    res = run_bass_kernel_spmd(nc, in_maps, list(range(b)))
    out = np.stack([res.results[i]["out"].reshape(c, h, w) for i in range(b)])
    return out.astype(np.float32)


if __name__ == "__main__":
    import time
    t0 = time.time()
    build()
    print(f"build ok in {time.time()-t0:.1f}s")
